# revision 1
# baseline (speedup 1.0000x reference)
"""Trainium2 Bass kernel for CoarseBlockAttention.

Reference computation (per batch b, with x: (C, H, W), C=512, H=W=64, S=4):
  x_avg  = 4x4 block means of x            -> (nb=256, C)  [unfold order bh*16+bw]
  Q = x_avg @ Wq.T + bq ; K = x_avg @ Wk.T + bk
  A = softmax(Q K^T / sqrt(C))             -> (256, 256)
  V = x_flat @ Wv.T + bv  (x_flat: flat row-major pixels, (4096, C))
  Vsum = V summed over groups of 16 consecutive flat pixels -> (256, C)
  out_small = A @ Vsum                     -> (256, C)
  out[c, p] = out_small[p // 16, c]        (repeat_interleave by 16)

Algebraic restructuring used here (all exact):
  * Vsum = Xsum @ Wv.T + 16*bv  with Xsum the group-of-16 pixel sums of x
    (linearity) -- shrinks the V projection by 16x.
  * Softmax rows of A sum to 1 => A @ (1 (16 bv)^T) = 1 (16 bv)^T, so the V
    bias is a per-channel constant added to out_small at the end.
  * Q K^T = xa (Wq^T Wk) xa^T + [row-const terms] + 1 (u . xa[m])^T with
    u = Wk^T bq.  Row-constant terms cancel in softmax.  So only the fused
    matrix W2 = Wq^T Wk and vector u are needed; bq/bk never materialize.
  * The 1/16 block-mean scaling and 1/sqrt(C) logit scaling are folded into
    W2 and u on the host.

Device layout (per core = one batch element, 8 cores data-parallel over B=8):
  XaT[c, n] : 4x4 block sums   (C on partitions, 4 chunks of 128)
  XsT[c, m] : 1x16 run sums    (same layout)
  G = W2s @ XaT        (PE, contracting c' chunks)       -> (c, 256)
  L = XaT^T G + 1 cs^T (PE)                              -> (n, 256) logits
  A = softmax rows (DVE reduce max / ACT exp / DVE reciprocal+scale)
  At = A^T (PE transpose)                                 -> (m, n)
  Vs = XsT^T WvT       (PE)                              -> (m, o=512)
  outT = Vs^T At  (PE) -> (o, n); ACT adds 16*bv and expands 16x along free
  dim (broadcast read from PSUM) before the contiguous DMA store.
"""

import math
from contextlib import ExitStack

import numpy as np

import concourse.bacc as bacc
import concourse.bass as bass
import concourse.mybir as mybir
import concourse.tile as tile
from concourse._compat import get_trn_type
from concourse.bass_utils import run_bass_kernel_spmd
from concourse.masks import make_identity

B, C, H, W, S = 8, 512, 64, 64, 4
HW = H * W          # 4096
NB = (H // S) * (W // S)  # 256
P = 128
KC = C // P         # 4 contraction/channel chunks
F32 = mybir.dt.float32
AX = mybir.AxisListType
AF = mybir.ActivationFunctionType


def _kernel_body(tc: "tile.TileContext", ctx, out, xb, w2t, wvt, us, b16):
    nc = tc.nc
    # fp32r: 1 cycle/row on PE (vs 4 for fp32).  walrus requires every fp32r
    # matmul operand to be *produced* with dtype float32r, so the operand
    # tiles are declared float32r and the producing engine rounds on write.
    FR = mybir.dt.float32r
    r = lambda ap: ap

    singles = ctx.enter_context(tc.tile_pool(name="singles", bufs=1))
    xpool = ctx.enter_context(tc.tile_pool(name="xpool", bufs=3))
    s1pool = ctx.enter_context(tc.tile_pool(name="s1pool", bufs=2))
    prpool = ctx.enter_context(tc.tile_pool(name="prpool", bufs=2))
    expool = ctx.enter_context(tc.tile_pool(name="expool", bufs=2))

    # Warm the ACT exp table during the DMA-in phase.
    dummy = singles.tile([P, 1], F32, name="dummy")
    nc.vector.memset(dummy, 0.0)
    nc.scalar.activation(dummy, dummy, AF.Exp)

    ident = singles.tile([P, P], F32, name="ident")
    make_identity(nc, ident)
    ones1_f = singles.tile([1, P], F32, name="ones1_f")
    nc.vector.memset(ones1_f, 1.0)
    ones1 = singles.tile([1, P], FR, name="ones1")
    nc.vector.tensor_copy(ones1, ones1_f)

    w2_sb = singles.tile([P, KC, C], FR, name="w2_sb")
    wv_sb = singles.tile([P, KC, C], FR, name="wv_sb")
    w2_d = w2t.rearrange("(k p) c -> p k c", p=P)
    wv_d = wvt.rearrange("(k p) c -> p k c", p=P)
    us_sb = singles.tile([P, KC], FR, name="us_sb")
    b16_sb = singles.tile([P, KC], F32, name="b16_sb")

    xa_sb = singles.tile([P, KC, NB], FR, name="xa_sb")  # 4x4 block sums^T
    xs_sb = singles.tile([P, KC, NB], FR, name="xs_sb")  # 1x16 run sums^T

    psA = tc.alloc_tile_pool(name="psA", bufs=1, space="PSUM")
    g_ps = [psA.tile([P, NB], F32, name=f"g_ps{j}") for j in range(KC)]
    vs_ps = [psA.tile([P, C], F32, name=f"vs_ps{m}") for m in range(2)]
    cs_ps = psA.tile([1, NB], F32, name="cs_ps")

    # Streaming phase: x arrives in 1 MB half-chunk pieces; pairwise-add trees
    # produce the 4-wide sums (DVE takes piece h=0, GPSIMD piece h=1, so the
    # two engines chase the DMA stream in parallel).  Weight slices are
    # interleaved between x pieces so they don't delay the first reductions.
    PW = HW // 2  # 2048 columns per piece
    for k in range(KC):
        s1 = s1pool.tile([P, 1024], F32, name="s1")
        for h in range(2):
            x_t = xpool.tile([P, PW], F32, name="x_t")
            nc.sync.dma_start(
                out=x_t, in_=xb[k * P:(k + 1) * P, h * PW:(h + 1) * PW]
            )
            eng = nc.vector if h == 0 else nc.gpsimd
            xv = x_t.rearrange("p (q two) -> p q two", two=2)
            pr = prpool.tile([P, 1024], F32, name="pr")
            eng.tensor_add(pr, xv[:, :, 0], xv[:, :, 1])
            pv = pr.rearrange("p (q two) -> p q two", two=2)
            eng.tensor_add(s1[:, h * 512:(h + 1) * 512], pv[:, :, 0], pv[:, :, 1])
        if k == 0:
            nc.sync.dma_start(out=us_sb, in_=us.rearrange("(k p) -> p k", p=P))
            nc.sync.dma_start(out=b16_sb, in_=b16.rearrange("(k p) -> p k", p=P))
        # weight slices for this chunk's matmuls (and spares) land here
        nc.sync.dma_start(out=w2_sb[:, k, :], in_=w2_d[:, k, :])
        nc.sync.dma_start(out=wv_sb[:, k, :], in_=wv_d[:, k, :])
        with nc.allow_low_precision(reason="fp32r matmul operands"):
            # 1x16 run sums: 4 consecutive s1 entries (same h)
            nc.vector.reduce_sum(
                xs_sb[:, k, :], s1.rearrange("p (m r) -> p m r", r=4), axis=AX.X
            )
            # 4x4 block sums: 4 s1 entries strided by 16 (dh direction)
            nc.vector.reduce_sum(
                xa_sb[:, k, :],
                s1.rearrange("p (bh dh bw) -> p bh bw dh", dh=4, bw=16),
                axis=AX.X,
            )
        first, last = (k == 0), (k == KC - 1)
        for j in range(KC):
            nc.tensor.matmul(
                g_ps[j],
                lhsT=r(w2_sb[:, k, j * P:(j + 1) * P]),
                rhs=r(xa_sb[:, k, :]),
                start=first,
                stop=last,
            )
        for m in range(2):
            nc.tensor.matmul(
                vs_ps[m],
                lhsT=r(xs_sb[:, k, m * P:(m + 1) * P]),
                rhs=r(wv_sb[:, k, :]),
                start=first,
                stop=last,
            )
        nc.tensor.matmul(
            cs_ps,
            lhsT=r(us_sb[:, k:k + 1]),
            rhs=r(xa_sb[:, k, :]),
            start=first,
            stop=last,
        )

    # PSUM -> SBUF staging, split across ACT and DVE to cut the latency on the
    # critical path into the L matmuls.
    g_sb = singles.tile([P, KC, NB], FR, name="g_sb")
    for j in range(KC):
        if j < 2:
            nc.scalar.copy(g_sb[:, j, :], g_ps[j])
        else:
            nc.vector.tensor_copy(g_sb[:, j, :], g_ps[j])
    vs_sb = singles.tile([P, 2, C], FR, name="vs_sb")
    nc.scalar.copy(vs_sb[:, 0, :], vs_ps[0])
    nc.vector.tensor_copy(vs_sb[:, 1, :], vs_ps[1])
    cs_sb = singles.tile([1, NB], FR, name="cs_sb")
    nc.scalar.copy(cs_sb, cs_ps)
    psA.release()

    psB = tc.alloc_tile_pool(name="psB", bufs=1, space="PSUM")

    # Logits + softmax (row chunks of 128).
    a_sb = singles.tile([P, 2, NB], F32, name="a_sb")
    nmax = singles.tile([P, 2], F32, name="nmax")
    rsum = singles.tile([P, 2], F32, name="rsum")
    l_ps = [psB.tile([P, NB], F32, name=f"l_ps{n}") for n in range(2)]
    for n in range(2):
        for k in range(KC):
            nc.tensor.matmul(
                l_ps[n],
                lhsT=r(xa_sb[:, k, n * P:(n + 1) * P]),
                rhs=r(g_sb[:, k, :]),
                start=(k == 0),
                stop=False,
            )
        # + 1 cs^T : broadcast the column-bias row via a K=1 matmul
        nc.tensor.matmul(
            l_ps[n], lhsT=r(ones1), rhs=r(cs_sb), start=False, stop=True
        )
        nc.vector.reduce_max(nmax[:, n:n + 1], l_ps[n], axis=AX.X, negate=True)
        nc.scalar.activation(
            a_sb[:, n, :],
            l_ps[n],
            AF.Exp,
            bias=nmax[:, n:n + 1],
            accum_out=rsum[:, n:n + 1],
        )
        nc.vector.reciprocal(rsum[:, n:n + 1], rsum[:, n:n + 1])
        nc.vector.tensor_scalar_mul(a_sb[:, n, :], a_sb[:, n, :], rsum[:, n:n + 1])

    # At[m, n] = A[n, m] via PE transpose of 128x128 blocks.
    at_sb = singles.tile([P, 2, NB], FR, name="at_sb")
    for n in range(2):
        for m in range(2):
            t_ps = psB.tile([P, P], F32, name="t_ps", bufs=2)
            nc.tensor.transpose(t_ps, a_sb[:, n, m * P:(m + 1) * P], ident)
            nc.vector.tensor_copy(at_sb[:, m, n * P:(n + 1) * P], t_ps)

    # outT[o, n] = sum_m Vs[m, o] At[m, n]; then +16*bv and 16x expansion.
    o_ps = [psB.tile([P, NB], F32, name=f"o_ps{j}") for j in range(KC)]
    for j in range(KC):
        for m in range(2):
            nc.tensor.matmul(
                o_ps[j],
                lhsT=r(vs_sb[:, m, j * P:(j + 1) * P]),
                rhs=r(at_sb[:, m, :]),
                start=(m == 0),
                stop=(m == 1),
            )
        ex = expool.tile([P, HW], F32, name="ex")
        nc.scalar.activation(
            ex.rearrange("p (q s) -> p q s", s=16),
            o_ps[j].broadcast_to((P, NB, 16)),
            AF.Identity,
            bias=b16_sb[:, j:j + 1],
        )
        nc.sync.dma_start(out=out[j * P:(j + 1) * P, :], in_=ex)
    psB.release()


def _build():
    nc = bacc.Bacc(
        get_trn_type() or "TRN2", target_bir_lowering=False, debug=False
    )
    xb = nc.dram_tensor("xb", (C, HW), F32, kind="ExternalInput").ap()
    w2t = nc.dram_tensor("w2t", (C, C), mybir.dt.float32r, kind="ExternalInput").ap()
    wvt = nc.dram_tensor("wvt", (C, C), mybir.dt.float32r, kind="ExternalInput").ap()
    us = nc.dram_tensor("us", (C,), mybir.dt.float32r, kind="ExternalInput").ap()
    b16 = nc.dram_tensor("b16", (C,), F32, kind="ExternalInput").ap()
    out = nc.dram_tensor("out", (C, HW), F32, kind="ExternalOutput").ap()

    with tile.TileContext(nc) as tc:
        with ExitStack() as ctx:
            _kernel_body(tc, ctx, out, xb, w2t, wvt, us, b16)
    nc.compile()
    return nc


_CACHE: dict = {}


def _get_nc():
    if "nc" not in _CACHE:
        _CACHE["nc"] = _build()
    return _CACHE["nc"]


def _prep_inputs(x, Wq, bq, Wk, bk, Wv, bv):
    f = lambda a: np.ascontiguousarray(np.asarray(a, dtype=np.float32))
    x, Wq, bq, Wk, bk, Wv, bv = map(f, (x, Wq, bq, Wk, bk, Wv, bv))
    s = 1.0 / math.sqrt(C)
    w2t = np.ascontiguousarray((Wk.T @ Wq) * (s / 256.0)).astype(np.float32)
    usv = ((Wk.T @ bq) * (s / 16.0)).astype(np.float32)
    wvt = np.ascontiguousarray(Wv.T).astype(np.float32)
    b16 = (16.0 * bv).astype(np.float32)
    in_maps = [
        {
            "xb": np.ascontiguousarray(x[b].reshape(C, HW)),
            "w2t": w2t,
            "wvt": wvt,
            "us": usv,
            "b16": b16,
        }
        for b in range(B)
    ]
    return in_maps


def run(inputs: dict, trace: bool = False, tmpdir: str | None = None):
    """Run on 8 NeuronCores; returns (output (B,C,H,W) f32, BassKernelResults)."""
    nc = _get_nc()
    in_maps = _prep_inputs(**inputs)
    rr = run_bass_kernel_spmd(nc, in_maps, list(range(B)), trace=trace, tmpdir=tmpdir)
    out = np.stack([r["out"] for r in rr.results]).reshape(B, C, H, W)
    return out.astype(np.float32), rr


def kernel(**inputs) -> np.ndarray:
    out, _ = run(inputs, trace=False)
    return out



# revision 7
# speedup vs baseline: 1.4499x; 1.4499x over previous
"""Trainium2 Bass kernel for CoarseBlockAttention (fp16 I/O pipeline).

Reference computation (per batch b, with x: (C, H, W), C=512, H=W=64, S=4):
  x_avg  = 4x4 block means of x            -> (nb=256, C)  [unfold order bh*16+bw]
  Q = x_avg @ Wq.T + bq ; K = x_avg @ Wk.T + bk
  A = softmax(Q K^T / sqrt(C))             -> (256, 256)
  V = x_flat @ Wv.T + bv  (x_flat: flat row-major pixels, (4096, C))
  Vsum = V summed over groups of 16 consecutive flat pixels -> (256, C)
  out_small = A @ Vsum                     -> (256, C)
  out[c, p] = out_small[p // 16, c]        (repeat_interleave by 16)

Algebraic restructuring (exact, same as the fp32 baseline):
  * Vsum = Xsum @ Wv.T + 16*bv (linearity); the V bias is a constant column
    added at the end (softmax rows sum to 1).
  * Q K^T row-constant terms cancel in softmax; only W2 = Wq^T Wk (fused,
    pre-scaled) and u = Wk^T bq survive.  1/16 and 1/sqrt(C) folded on host.

Performance structure (per core = one batch, 8 cores data-parallel over B=8):
  * All HBM I/O in fp16 (tolerance is 2e-2; measured end-to-end err ~6e-4):
    x in = 4 MB, weights 1 MB, out = 4 MB.
  * Host pre-permutes x columns so every reduction level is an fp16 add of
    two CONTIGUOUS halves (DVE 2x_1p mode): piece layout (clh, cll, r, c4).
  * Stream in 8 pieces (k channel chunk x h column half).  Per piece:
    two halving adds -> s1 (4-col sums), two small adds each -> Xsum / Xa
    in (r,q) / (bh,bw) order, then fp16 matmuls (G, Vs, cs) accumulate in
    PSUM.  Piece (3,h) triggers the m-half-h G copies + L matmuls so only
    ~half the attention chain remains after the last input byte.
  * Softmax (DVE max / ACT exp+accum / DVE recip+scale), PE transposes of A
    (fp16, 1 cyc/row), out matmuls per (j, n-half), ACT adds 16*bv from
    PSUM, DVE broadcast-copies the 16x expansion (2x_2p), fp16 DMA out in
    8 x 0.5 MB chunks so the store stream starts as early as possible.
"""

import math
from contextlib import ExitStack

import numpy as np

import concourse.bacc as bacc
import concourse.bass as bass
import concourse.mybir as mybir
import concourse.tile as tile
from concourse._compat import get_trn_type
from concourse.bass_utils import run_bass_kernel_spmd
from concourse.masks import make_identity

B, C, H, W, S = 8, 512, 64, 64, 4
HW = H * W                # 4096
NB = (H // S) * (W // S)  # 256
P = 128
KC = C // P               # 4 channel chunks
PW = HW // 2              # 2048 columns per (k, h) piece
F32 = mybir.dt.float32
F16 = mybir.dt.float16
AX = mybir.AxisListType
AF = mybir.ActivationFunctionType


def _kernel_body(tc: "tile.TileContext", ctx, out, xb, w2p, wvp, usp, b16p):
    nc = tc.nc

    singles = ctx.enter_context(tc.tile_pool(name="singles", bufs=1))
    xpool = ctx.enter_context(tc.tile_pool(name="xpool", bufs=3))
    tpool = ctx.enter_context(tc.tile_pool(name="tpool", bufs=2))
    s1pool = ctx.enter_context(tc.tile_pool(name="s1pool", bufs=2))
    uvpool = ctx.enter_context(tc.tile_pool(name="uvpool", bufs=2))
    expool = ctx.enter_context(tc.tile_pool(name="expool", bufs=3))

    # Warm the ACT exp table while DMAs stream in.
    dummy = singles.tile([P, 1], F32, name="dummy")
    nc.vector.memset(dummy, 0.0)
    nc.scalar.activation(dummy, dummy, AF.Exp)

    ident = singles.tile([P, P], F16, name="ident")
    make_identity(nc, ident)
    ones1 = singles.tile([1, P], F16, name="ones1")
    nc.vector.memset(ones1, 1.0)

    w2_sb = singles.tile([P, KC, C], F16, name="w2_sb")
    wv_sb = singles.tile([P, KC, C], F16, name="wv_sb")
    us_sb = singles.tile([P, KC], F16, name="us_sb")
    b16_sb = singles.tile([P, KC], F32, name="b16_sb")

    xa_sb = singles.tile([P, KC, NB], F16, name="xa_sb")  # 4x4 block sums^T
    xs_sb = singles.tile([P, KC, NB], F16, name="xs_sb")  # 1x16 run sums^T
    g_sb = singles.tile([P, KC, NB], F16, name="g_sb")
    vs_sb = singles.tile([P, 2, C], F16, name="vs_sb")
    cs_sb = singles.tile([1, NB], F16, name="cs_sb")
    a_sb = singles.tile([P, 2, NB], F16, name="a_sb")
    at_sb = singles.tile([P, 2, NB], F16, name="at_sb")
    os_sb = singles.tile([P, KC, NB], F16, name="os_sb")
    nmax = singles.tile([P, 2], F32, name="nmax")
    rsum = singles.tile([P, 2], F32, name="rsum")

    # PSUM is bank-granular (2 KB/partition per bank, 8 banks): pack pairs of
    # (P, 256) f32 regions into single 1-bank tiles.  psL outlives psA (pool
    # release is LIFO), so allocate it first.
    psL = tc.alloc_tile_pool(name="psL", bufs=1, space="PSUM")
    l2_ps = psL.tile([P, 2, NB], F32, name="l2_ps")
    l_ps = [l2_ps[:, n, :] for n in range(2)]
    t2_ps = psL.tile([P, 2, P], F16, name="t2_ps")
    psA = tc.alloc_tile_pool(name="psA", bufs=1, space="PSUM")
    g2_ps = [psA.tile([P, 2, NB], F32, name=f"g2_ps{i}") for i in range(2)]
    g_ps = [g2_ps[j // 2][:, j % 2, :] for j in range(KC)]
    vs_ps = [psA.tile([P, C], F32, name=f"vs_ps{m}") for m in range(2)]
    cs_ps = psA.tile([1, NB], F32, name="cs_ps")

    # Weights on the scalar (ACT) HWDGE queue so they don't stall the x
    # stream on the sync queue.
    nc.scalar.dma_start(out=w2_sb, in_=w2p.rearrange("p (k c) -> p k c", c=C))
    nc.scalar.dma_start(out=wv_sb, in_=wvp.rearrange("p (k c) -> p k c", c=C))
    nc.scalar.dma_start(out=us_sb, in_=usp)
    nc.scalar.dma_start(out=b16_sb, in_=b16p)

    def half_tail(h):
        """After piece (KC-1, h): stage G/cs/Vs m-half h and run L matmuls."""
        mr = slice(h * P, (h + 1) * P)
        for j in range(KC):
            if j < 2:
                nc.scalar.copy(g_sb[:, j, mr], g_ps[j][:, mr])
            else:
                nc.vector.tensor_copy(g_sb[:, j, mr], g_ps[j][:, mr])
        nc.scalar.copy(cs_sb[:, mr], cs_ps[:, mr])
        nc.vector.tensor_copy(vs_sb[:, h, :], vs_ps[h])
        for n in range(2):
            for kk in range(KC):
                nc.tensor.matmul(
                    l_ps[n][:, mr],
                    lhsT=xa_sb[:, kk, n * P:(n + 1) * P],
                    rhs=g_sb[:, kk, mr],
                    start=(kk == 0),
                    stop=False,
                )
            nc.tensor.matmul(
                l_ps[n][:, mr], lhsT=ones1, rhs=cs_sb[:, mr],
                start=False, stop=True,
            )

    with nc.allow_low_precision(reason="fp16 pipeline (tolerance 2e-2)"):
        for k in range(KC):
            for h in range(2):
                nr = slice(h * P, (h + 1) * P)
                x_t = xpool.tile([P, PW], F16, name="x_t")
                nc.sync.dma_start(
                    out=x_t, in_=xb[k * P:(k + 1) * P, h * PW:(h + 1) * PW]
                )
                # contiguous-half adds: (clh cll r c4) -> s1 (r c4)
                t = tpool.tile([P, 1024], F16, name="t")
                nc.vector.tensor_add(t, x_t[:, 0:1024], x_t[:, 1024:2048])
                s1 = s1pool.tile([P, 512], F16, name="s1")
                nc.vector.tensor_add(s1, t[:, 0:512], t[:, 512:1024])
                # Xsum: m = 4r + q, sum over cq (innermost pairs)
                s1m = s1.rearrange("p (m c) -> p m c", c=4)
                u = uvpool.tile([P, P, 2], F16, name="u")
                nc.vector.tensor_add(u, s1m[:, :, 0:2], s1m[:, :, 2:4])
                nc.vector.tensor_add(xs_sb[:, k, nr], u[:, :, 0], u[:, :, 1])
                # Xa: n = 16 bh + c4, sum over dh (stride-16 halves)
                s1b = s1.rearrange("p (bh dh c) -> p bh dh c", dh=4, c=16)
                v = uvpool.tile([P, 8, 2, 16], F16, name="v")
                nc.vector.tensor_add(v, s1b[:, :, 0:2, :], s1b[:, :, 2:4, :])
                nc.vector.tensor_add(
                    xa_sb[:, k, nr].rearrange("p (bh c) -> p bh c", c=16),
                    v[:, :, 0, :], v[:, :, 1, :],
                )
                first, last = (k == 0), (k == KC - 1)
                for j in range(KC):
                    nc.tensor.matmul(
                        g_ps[j][:, nr],
                        lhsT=w2_sb[:, k, j * P:(j + 1) * P],
                        rhs=xa_sb[:, k, nr],
                        start=first,
                        stop=last,
                    )
                nc.tensor.matmul(
                    vs_ps[h],
                    lhsT=xs_sb[:, k, nr],
                    rhs=wv_sb[:, k, :],
                    start=first,
                    stop=last,
                )
                nc.tensor.matmul(
                    cs_ps[:, nr],
                    lhsT=us_sb[:, k:k + 1],
                    rhs=xa_sb[:, k, nr],
                    start=first,
                    stop=last,
                )
                if last:
                    half_tail(h)

        psA.release()
        psO = tc.alloc_tile_pool(name="psO", bufs=1, space="PSUM")
        o2_ps = [psO.tile([P, 2, NB], F32, name=f"o2_ps{i}") for i in range(2)]
        o_ps = [o2_ps[j // 2][:, j % 2, :] for j in range(KC)]

        def emit_out(j, nh):
            nhr = slice(nh * P, (nh + 1) * P)
            for m in range(2):
                nc.tensor.matmul(
                    o_ps[j][:, nhr],
                    lhsT=vs_sb[:, m, j * P:(j + 1) * P],
                    rhs=at_sb[:, m, nhr],
                    start=(m == 0),
                    stop=(m == 1),
                )
            nc.scalar.activation(
                os_sb[:, j, nhr], o_ps[j][:, nhr], AF.Identity,
                bias=b16_sb[:, j:j + 1],
            )
            ex = expool.tile([P, P, 16], F16, name="ex")
            nc.vector.tensor_copy(ex, os_sb[:, j, nhr].broadcast_to((P, P, 16)))
            nc.sync.dma_start(
                out=out[j * P:(j + 1) * P, nh * PW:(nh + 1) * PW],
                in_=ex.rearrange("p q s -> p (q s)"),
            )

        # Softmax + transpose per row-half; first output chunk right after n=0.
        for n in range(2):
            nc.vector.reduce_max(nmax[:, n:n + 1], l_ps[n], axis=AX.X, negate=True)
            nc.scalar.activation(
                a_sb[:, n, :], l_ps[n], AF.Exp,
                bias=nmax[:, n:n + 1], accum_out=rsum[:, n:n + 1],
            )
            nc.vector.reciprocal(rsum[:, n:n + 1], rsum[:, n:n + 1])
            nc.vector.tensor_scalar_mul(
                a_sb[:, n, :], a_sb[:, n, :], rsum[:, n:n + 1]
            )
            for m in range(2):
                t_ps = t2_ps[:, m, :]
                nc.tensor.transpose(t_ps, a_sb[:, n, m * P:(m + 1) * P], ident)
                nc.vector.tensor_copy(at_sb[:, m, n * P:(n + 1) * P], t_ps)
            if n == 0:
                emit_out(0, 0)
        for j in range(1, KC):
            emit_out(j, 0)
        for j in range(KC):
            emit_out(j, 1)
        psO.release()
        psL.release()


def _build():
    nc = bacc.Bacc(
        get_trn_type() or "TRN2", target_bir_lowering=False, debug=False
    )
    xb = nc.dram_tensor("xb", (C, HW), F16, kind="ExternalInput").ap()
    w2p = nc.dram_tensor("w2p", (P, KC * C), F16, kind="ExternalInput").ap()
    wvp = nc.dram_tensor("wvp", (P, KC * C), F16, kind="ExternalInput").ap()
    usp = nc.dram_tensor("usp", (P, KC), F16, kind="ExternalInput").ap()
    b16p = nc.dram_tensor("b16p", (P, KC), F32, kind="ExternalInput").ap()
    out = nc.dram_tensor("out", (C, HW), F16, kind="ExternalOutput").ap()

    with tile.TileContext(nc) as tc:
        with ExitStack() as ctx:
            _kernel_body(tc, ctx, out, xb, w2p, wvp, usp, b16p)
    nc.compile()
    return nc


_CACHE: dict = {}


def _get_nc():
    if "nc" not in _CACHE:
        _CACHE["nc"] = _build()
    return _CACHE["nc"]


def _prep_inputs(x, Wq, bq, Wk, bk, Wv, bv):
    f = lambda a: np.ascontiguousarray(np.asarray(a, dtype=np.float32))
    x, Wq, bq, Wk, bk, Wv, bv = map(f, (x, Wq, bq, Wk, bk, Wv, bv))
    s = 1.0 / math.sqrt(C)
    w2t = (Wk.T @ Wq) * (s / 256.0)           # (c', c); lhsT for G
    usv = (Wk.T @ bq) * (s / 16.0)
    wvt = Wv.T                                # (c', o); rhs for Vs
    b16 = (16.0 * bv).astype(np.float32)
    w2p = np.ascontiguousarray(
        w2t.reshape(KC, P, C).transpose(1, 0, 2).reshape(P, KC * C)
    ).astype(np.float16)
    wvp = np.ascontiguousarray(
        wvt.reshape(KC, P, C).transpose(1, 0, 2).reshape(P, KC * C)
    ).astype(np.float16)
    usp = np.ascontiguousarray(usv.reshape(KC, P).T).astype(np.float16)
    b16p = np.ascontiguousarray(b16.reshape(KC, P).T)
    # piece layout (h | clh cll r c4): every reduction level is an add of two
    # contiguous halves on device.
    xd = np.ascontiguousarray(
        x.reshape(B, C, 2, 32, 16, 2, 2)
        .transpose(0, 1, 2, 5, 6, 3, 4)
        .reshape(B, C, HW)
    ).astype(np.float16)
    in_maps = [
        {"xb": xd[b], "w2p": w2p, "wvp": wvp, "usp": usp, "b16p": b16p}
        for b in range(B)
    ]
    return in_maps


def run(inputs: dict, trace: bool = False, tmpdir: str | None = None):
    """Run on 8 NeuronCores; returns (output (B,C,H,W) f32, BassKernelResults)."""
    nc = _get_nc()
    in_maps = _prep_inputs(**inputs)
    rr = run_bass_kernel_spmd(nc, in_maps, list(range(B)), trace=trace, tmpdir=tmpdir)
    out = np.stack([r["out"] for r in rr.results]).astype(np.float32)
    return out.reshape(B, C, H, W), rr


def kernel(**inputs) -> np.ndarray:
    out, _ = run(inputs, trace=False)
    return out
